# revision 59
# baseline (speedup 1.0000x reference)
"""Trainium2 Bass kernel for KernelAttention (gaussian-kernel multi-head attention).

Math (per batch b):
  d2[q,k]   = |q_pos[q] - k_pos[k]|^2   (as -d2 via one K=15 augmented matmul)
  s_h[k,q]  = exp(-c_h * d2),  c_h = 1/lengthscale_h^2
  att_h[q,v]= sum_k s_h[k,q] * V[k,h,v] / (sum_k s_h[k,q]*unmasked[k] + 1e-5)
  out[q,o]  = sum_{h,v} att_h[q,v] * w_out[o, h*64+v]

Sharding: 8 cores = (batch b in 0..3) x (query half in 0..1); each core owns
[1024 q, all keys].  All inputs host-prepped per core; outputs host-gathered.

Key structure (vs. a dense implementation):
- Masked keys are compacted away on the host (~half the keys), shrinking the
  k extent from 16 tiles to KT2 (9 here) tiles.  Padded tail keys carry
  zeroed values/ones-column so they contribute nothing.
- Scores stay transposed [k, q] so the attend matmul (lhsT = values+ones col,
  rhs = scores) contracts k on the PE partition dim; psum row 64 accumulates
  the normalizer via the mask-zeroed ones column.
- m = -d2 is evacuated from PSUM to fp32 SBUF, so every exp reads
  full-precision distances (bf16-m rounding amplified by c_h=100 was the
  baseline's dominant error).
- Large-lengthscale heads (c_h*d2max small) are factored through a rank-70
  polynomial basis: s_h ~= U[q,:] @ W_h[k,:]^T with monomial features of the
  augmented position vectors (host-computed).  Their attend collapses to
  G_h = W_h^T V_h (tiny) and att_h = G_h^T @ U^T, skipping both the exp and
  the O(k*q) attend matmuls.
- One moderate head is chained as s_d = (s_src^2)^2 on DVE, shortening the
  ACT exp critical path that paces the attends.
- Normalization: eps-add fused into the norm-row evacuation; early pairs
  batched into one [6,q] fast reciprocal + K=6 selection matmuls emitted
  after the last attend; the final pair uses per-head reciprocal + GPSIMD
  partition_broadcast so its latency hides under the projection chunks.
"""

import numpy as np
import itertools
from math import factorial
from contextlib import ExitStack

B, LQ, LK, DPOS = 4, 2048, 2048, 3
H, V, OUTD = 8, 64, 512
QS = LQ // 2          # q rows per core
V1 = V + 1            # value cols + ones col
NCORES = 8
DMAX = 64.0           # poly fit domain [0, DMAX] for d2
PMAX = 4
FEATS = [f for f in itertools.product(range(PMAX + 1), repeat=4)
         if sum(f) <= PMAX]
R = len(FEATS)        # 70 poly features

_cache = {}


def _cheb_power_coeffs(c, deg, tmax):
    """exp(-c*t) ~= sum_j bt[j] t^j on [0, tmax]; returns (bt, max_err)."""
    from numpy.polynomial import chebyshev as C, polynomial as P
    t = (np.cos(np.pi * (np.arange(4000) + 0.5) / 4000) + 1) / 2 * tmax
    f = np.exp(-np.float64(c) * t)
    ch = C.chebfit(t / tmax * 2 - 1, f, deg)
    bpow = C.cheb2poly(ch)
    bt = np.zeros(deg + 1)
    acc = np.array([1.0])
    lin = np.array([-1.0, 2.0 / tmax])
    for j in range(deg + 1):
        bt[:len(acc)] += bpow[j] * acc
        acc = P.polymul(acc, lin)
    err = np.abs(np.polyval(bt[::-1], t) - f).max()
    return bt, err


def _classify(cv):
    """Heads -> poly-factorizable / exp; pick one chained head (s_d=s_src^4)."""
    poly = {}
    for h in range(H):
        for deg in range(2, PMAX + 1):
            bt, err = _cheb_power_coeffs(cv[h], deg, DMAX)
            if err < 6e-4:
                poly[h] = (deg, bt)
                break
    exp_heads = [h for h in range(H) if h not in poly]
    chain = None
    for dd in exp_heads:
        for ss in exp_heads:
            if dd != ss and cv[dd] <= 2.0 and \
                    np.float32(cv[dd]) == np.float32(4.0) * np.float32(cv[ss]):
                chain = (dd, ss)
                break
        if chain:
            break
    return poly, exp_heads, chain


def _plan(cv):
    poly, exp_heads, chain = _classify(cv)
    cd = chain[0] if chain else None
    stream = [h for h in exp_heads if h != cd]
    last = cd if cd is not None else stream[-1]
    last_j = last // 2
    early_heads = sorted(h for h in range(H) if h // 2 != last_j)
    early_pairs = sorted({h // 2 for h in early_heads})
    row_of = {h: i for i, h in enumerate(early_heads)}
    return poly, exp_heads, chain, stream, last, last_j, early_heads, \
        early_pairs, row_of


def _build(cv, KT2):
    key = (tuple(cv), KT2)
    if key in _cache:
        return _cache[key]
    import concourse.bacc as bacc
    import concourse.tile as tile
    from concourse import mybir

    f32 = mybir.dt.float32
    bf16 = mybir.dt.bfloat16
    AF = mybir.ActivationFunctionType

    (poly, exp_heads, chain, stream, last, last_j, early_heads,
     early_pairs, row_of) = _plan(cv)
    n_poly = len(poly)
    poly_list = sorted(poly)
    NK = KT2 * 128
    NEP = len(early_pairs)

    nc = bacc.Bacc("TRN2", target_bir_lowering=False, debug=False,
                   num_devices=NCORES)
    # ka/qa: hi/lo bf16 split of the K=5 augmented distance operands:
    # rows [hi(5); lo(5); hi(5)] x [hi(5); hi(5); lo(5)] accumulate
    # hi*hi + lo*hi + hi*lo in fp32 PSUM in one K=15 matmul.
    ka = nc.dram_tensor("ka", [15, NK], bf16, kind="ExternalInput").ap()
    qa = nc.dram_tensor("qa", [15, QS], bf16, kind="ExternalInput").ap()
    vp = nc.dram_tensor("vp", [128, KT2, H * V1], bf16, kind="ExternalInput").ap()
    wt = nc.dram_tensor("wt", [128, 4, OUTD], bf16, kind="ExternalInput").ap()
    sel6 = nc.dram_tensor("sel6", [len(early_heads), NEP, 128], bf16,
                          kind="ExternalInput").ap()
    if n_poly:
        ut = nc.dram_tensor("ut", [R, QS], bf16, kind="ExternalInput").ap()
        wp = nc.dram_tensor("wp", [128, KT2, n_poly * R], bf16,
                            kind="ExternalInput").ap()
    outT = nc.dram_tensor("outT", [OUTD, QS], f32, kind="ExternalOutput").ap()

    NEH = len(early_heads)

    with tile.TileContext(nc) as tc, ExitStack() as ctx:
        const = ctx.enter_context(tc.tile_pool(name="const", bufs=1))
        # 4 bufs for 5 score tensors: the 5th tensor's writes start well
        # after the 1st head's attend has been evacuated, so it reuses buf 0
        spool = ctx.enter_context(tc.tile_pool(name="spool", bufs=4))
        gpool = ctx.enter_context(tc.tile_pool(name="gpool", bufs=2))
        stage = ctx.enter_context(tc.tile_pool(name="stage", bufs=2))
        rpool = ctx.enter_context(tc.tile_pool(name="rpool", bufs=2))
        obuf = ctx.enter_context(tc.tile_pool(name="obuf", bufs=2))
        psp = ctx.enter_context(tc.tile_pool(name="psum", bufs=4, space="PSUM"))

        ka_sb = const.tile([15, NK], bf16)
        nc.sync.dma_start(out=ka_sb[:], in_=ka)
        qa_sb = const.tile([15, QS], bf16)
        nc.sync.dma_start(out=qa_sb[:], in_=qa)
        vp_sb = const.tile([128, KT2, H * V1], bf16)
        nc.sync.dma_start(out=vp_sb[:], in_=vp)
        if n_poly:
            wp_sb = const.tile([128, KT2, n_poly * R], bf16)
            nc.sync.dma_start(out=wp_sb[:], in_=wp)
            ut_sb = const.tile([R, QS], bf16)
            nc.sync.dma_start(out=ut_sb[:], in_=ut)
        wt_sb = const.tile([128, 4, OUTD], bf16)
        nc.sync.dma_start(out=wt_sb[:], in_=wt)
        sel6_sb = const.tile([NEH, NEP, 128], bf16)
        nc.sync.dma_start(out=sel6_sb[:], in_=sel6)

        m_sb = const.tile([128, KT2, QS], f32)
        s_sb = {h: spool.tile([128, KT2, QS], bf16, tag="s", name=f"s{h}")
                for h in exp_heads}
        flat = [const.tile([128, QS], bf16, tag=f"flat{j}", name=f"flat{j}")
                for j in range(4)]
        norms6 = const.tile([NEH, QS], f32)
        norm_head = {}
        rb_last = {}

        # ---- Phase A: distance matmuls; evacuate -d2 to fp32 SBUF m ----
        for kt in range(KT2):
            d2 = psp.tile([128, QS], f32, tag="ps", name=f"d2_{kt}")
            for qc in range(2):
                s5 = slice(qc * 512, (qc + 1) * 512)
                nc.tensor.matmul(d2[:, s5],
                                 lhsT=ka_sb[:, kt * 128:(kt + 1) * 128],
                                 rhs=qa_sb[:, s5], start=True, stop=True)
            nc.vector.tensor_copy(out=m_sb[:, kt, :], in_=d2[:])

        # ---- ACT exp streams (per k-tile so attends can chase) ----
        for h in stream:
            for kt in range(KT2):
                nc.scalar.activation(out=s_sb[h][:, kt, :], in_=m_sb[:, kt, :],
                                     func=AF.Exp, scale=float(cv[h]))

        # evac: att psum -> flat rows; norm row + eps -> stage -> DMA out.
        # (PSUM reads are DVE-only: GPSIMD cannot access PSUM, ACT is busy.)
        def evac(att, h, on_act=False):
            j, r0 = h // 2, (h % 2) * 64
            stg = stage.tile([V1, QS], f32, tag="stg", name=f"stg{h}")
            if on_act:
                # ACT is idle once the exp streams finish; moving the last
                # attends' PSUM reads there unclogs the DVE tail
                nc.scalar.copy(out=flat[j][r0:r0 + 64, :], in_=att[0:64, :])
                nc.scalar.activation(out=stg[64:65, :], in_=att[64:65, :],
                                     func=AF.Copy, bias=1e-5)
                # issue the norm DMA from ACT's own DGE so it doesn't queue
                # behind gpsimd broadcasts
                if j == last_j:
                    nt = rpool.tile([1, QS], f32, tag="nl", name=f"nl{h}",
                                    bufs=2)
                    norm_head[h] = nt
                    nc.scalar.dma_start(out=nt[:], in_=stg[64:65, :])
                else:
                    rr = row_of[h]
                    nc.scalar.dma_start(out=norms6[rr:rr + 1, :],
                                        in_=stg[64:65, :])
                return
            else:
                nc.vector.tensor_copy(out=flat[j][r0:r0 + 64, :],
                                      in_=att[0:64, :])
                nc.vector.tensor_scalar_add(stg[64:65, :], att[64:65, :],
                                            1e-5)
            if j == last_j:
                nt = rpool.tile([1, QS], f32, tag="nl", name=f"nl{h}", bufs=2)
                norm_head[h] = nt
                nc.gpsimd.dma_start(out=nt[:], in_=stg[64:65, :])
            else:
                rr = row_of[h]
                nc.gpsimd.dma_start(out=norms6[rr:rr + 1, :],
                                    in_=stg[64:65, :])

        def attend(h, on_act=False, skip_evac=False):
            att = psp.tile([V1, QS], f32, tag="ps", name=f"att{h}")
            for qc in range(2):
                s5 = slice(qc * 512, (qc + 1) * 512)
                for kt in range(KT2):
                    nc.tensor.matmul(att[:, s5],
                                     lhsT=vp_sb[:, kt, h * V1:(h + 1) * V1],
                                     rhs=s_sb[h][:, kt, s5],
                                     start=(kt == 0), stop=(kt == KT2 - 1))
            if skip_evac:
                return att
            evac(att, h, on_act=on_act)

        # last pair: per-head fast reciprocal + GPSIMD broadcast as soon as
        # each head's normalizer lands (partner often completes early).
        def mark(h):
            if h // 2 != last_j:
                return
            nt = norm_head[h]
            nc.vector.reciprocal_approx_fast(out=nt[:], in_=nt[:])
            rb = rpool.tile([128, QS], f32, tag="rb", name=f"rbl{h}", bufs=2)
            nc.gpsimd.partition_broadcast(rb[:], nt[:], channels=128)
            rb_last[h] = rb

        # ---- PE schedule ----
        # defer the first attend's evacuation until after the first poly
        # g-copy: the tiny copy otherwise queues on DVE behind the evac ops
        # and stalls the PE's poly attends by ~4us
        att0_t = attend(stream[0], skip_evac=True)
        mark(stream[0])
        for i, h in enumerate(poly_list):
            G = psp.tile([R, V1], f32, tag="ps", name=f"G{h}")
            for kt in range(KT2):
                nc.tensor.matmul(G[:],
                                 lhsT=wp_sb[:, kt, i * R:(i + 1) * R],
                                 rhs=vp_sb[:, kt, h * V1:(h + 1) * V1],
                                 start=(kt == 0), stop=(kt == KT2 - 1))
            g_sb = gpool.tile([R, V1], bf16, tag="g", name=f"g{h}")
            nc.vector.tensor_copy(out=g_sb[:], in_=G[:])
            if att0_t is not None:
                evac(att0_t, stream[0])
                att0_t = None
            attp = psp.tile([V1, QS], f32, tag="ps", name=f"attp{h}")
            for qc in range(2):
                s5 = slice(qc * 512, (qc + 1) * 512)
                nc.tensor.matmul(attp[:, s5], lhsT=g_sb[:],
                                 rhs=ut_sb[:, s5], start=True, stop=True)
            evac(attp, h)
            mark(h)
        if att0_t is not None:
            evac(att0_t, stream[0])
            att0_t = None
        chain_emitted = False

        def emit_chain():
            cd, cs = chain
            for kt in range(KT2):
                t = gpool.tile([128, QS], bf16, tag="chain", name=f"ch{kt}")
                nc.vector.tensor_mul(t[:], s_sb[cs][:, kt, :],
                                     s_sb[cs][:, kt, :])
                nc.vector.tensor_mul(s_sb[cd][:, kt, :], t[:], t[:])

        for hi_, h in enumerate(stream[1:]):
            attend(h, on_act=(hi_ == len(stream[1:]) - 1))
            mark(h)
            # chained head s_d = (s_src^2)^2 on DVE: emit as early as
            # possible (the muls chase the source exp stream tile by tile)
            # so the last attend is never chain-paced and the DVE queue is
            # clear again before the normalization ops arrive
            if chain and not chain_emitted and hi_ == 0:
                emit_chain()
                chain_emitted = True
        if chain and not chain_emitted:
            emit_chain()

        # batched early-pair reciprocals (eps already folded in at evac)
        rhi6 = rpool.tile([NEH, QS], bf16, tag="rhi6")
        rlo6 = rpool.tile([NEH, QS], bf16, tag="rlo6")
        nc.vector.reciprocal_approx_fast(out=norms6[:], in_=norms6[:])
        nc.vector.tensor_copy(out=rhi6[:], in_=norms6[:])
        nc.vector.tensor_sub(rlo6[:], norms6[:], rhi6[:])
        attend(last, on_act=True)

        # early-pair rb via K=6 selection matmuls (inputs ready by now)
        for idx, j in enumerate(early_pairs):
            rbp = psp.tile([128, QS], f32, tag="ps", name=f"rbp{j}")
            for qc in range(2):
                s5 = slice(qc * 512, (qc + 1) * 512)
                nc.tensor.matmul(rbp[:, s5], lhsT=sel6_sb[:, idx, :],
                                 rhs=rhi6[:, s5], start=True, stop=False)
                nc.tensor.matmul(rbp[:, s5], lhsT=sel6_sb[:, idx, :],
                                 rhs=rlo6[:, s5], start=False, stop=True)
            nc.vector.tensor_mul(flat[j][:], flat[j][:], rbp[:])
        # last pair: reciprocal+broadcast for the final head, then halves
        # multiplied against the broadcast tiles
        mark(last)
        nc.vector.tensor_mul(flat[last_j][0:64, :], flat[last_j][0:64, :],
                             rb_last[2 * last_j][0:64, :])
        nc.vector.tensor_mul(flat[last_j][64:128, :], flat[last_j][64:128, :],
                             rb_last[2 * last_j + 1][64:128, :])

        # ---- out projection: outT[o, q] = sum_hv wt[hv, o] * flat[hv, q] ----
        po_t = {}

        def po_chunk(ot, j, start=False, stop=False):
            po = po_t.get(ot)
            if po is None:
                po = po_t[ot] = psp.tile([128, QS], f32, tag="ps",
                                         name=f"po{ot}", uniquify=True)
            for qc in range(2):
                s5 = slice(qc * 512, (qc + 1) * 512)
                nc.tensor.matmul(po[:, s5],
                                 lhsT=wt_sb[:, j, ot * 128:(ot + 1) * 128],
                                 rhs=flat[j][:, s5],
                                 start=start, stop=stop)

        # split the 2MB output across three DGE queues so the transfers
        # overlap instead of serializing on the sync queue
        dma_engs = [nc.scalar, nc.sync, nc.gpsimd, nc.sync]

        def po_evac(ot, eng):
            ob = obuf.tile([128, QS], f32, tag="ob", name=f"ob{ot}")
            if eng is nc.scalar:
                eng.copy(out=ob[:], in_=po_t[ot][:])
            else:
                eng.tensor_copy(out=ob[:], in_=po_t[ot][:])
            dma_engs[ot].dma_start(out=outT[ot * 128:(ot + 1) * 128, :],
                                   in_=ob[:])

        for ot in range(4):
            for jj, j in enumerate(early_pairs):
                po_chunk(ot, j, start=(jj == 0))
        ev_engs = [nc.scalar, nc.vector, nc.scalar, nc.vector]
        for ot in range(4):
            po_chunk(ot, last_j, stop=True)
            po_evac(ot, ev_engs[ot])

    nc.compile()
    _cache[key] = nc
    return nc


def _hilo(x, bf16):
    hi = x.astype(bf16)
    lo = (x - hi.astype(np.float32)).astype(bf16)
    return hi, lo


def _build_U(q):
    q2s = (q ** 2).sum(-1)
    return np.stack([(q2s ** a) * (q[:, 0] ** c1) * (q[:, 1] ** c2)
                     * (q[:, 2] ** c3) for a, c1, c2, c3 in FEATS], 1)


def _build_W(k, coeffs, deg):
    k2s = (k ** 2).sum(-1)
    cols = []
    for a, c1, c2, c3 in FEATS:
        cc = c1 + c2 + c3
        col = np.zeros(len(k))
        for j in range(a + cc, deg + 1):
            bb = j - a - cc
            mult = factorial(j) / (factorial(a) * factorial(bb)
                                   * factorial(c1) * factorial(c2)
                                   * factorial(c3))
            col += coeffs[j] * mult * ((-2.0) ** cc) * (k2s ** bb) \
                * (k[:, 0] ** c1) * (k[:, 1] ** c2) * (k[:, 2] ** c3)
        cols.append(col)
    return np.stack(cols, 1)


def _prep_batch(kp_b, vals_b, mask_b, KT2, poly, bf16):
    """Per-batch key-side prep: compact unmasked keys, pad to KT2*128."""
    NK = KT2 * 128
    idx = np.where(~mask_b)[0]
    nk = len(idx)
    kpos = np.zeros((NK, DPOS), np.float32)
    kpos[:nk] = kp_b[idx]
    k2 = (kpos * kpos).sum(-1)
    ones_pad = np.zeros(NK, np.float32)
    ones_pad[:nk] = 1.0
    ka5 = np.stack([kpos[:, 0], kpos[:, 1], kpos[:, 2], k2, ones_pad])
    ka_hi, ka_lo = _hilo(ka5.astype(np.float32), bf16)
    ka = np.concatenate([ka_hi, ka_lo, ka_hi])          # [15, NK]
    vv = np.zeros((NK, H, V1), np.float32)
    vv[:nk, :, :V] = vals_b[idx]
    vv[:nk, :, V] = 1.0
    vp = vv.reshape(KT2, 128, H * V1).transpose(1, 0, 2).astype(bf16)
    wp = None
    if poly:
        wcols = []
        for h in sorted(poly):
            deg, coeffs = poly[h]
            W = _build_W(kpos.astype(np.float64), coeffs, deg)
            W[nk:] = 0.0
            wcols.append(W.astype(np.float32))
        Wall = np.concatenate(wcols, 1)                 # [NK, n_poly*R]
        wp = Wall.reshape(KT2, 128, -1).transpose(1, 0, 2).astype(bf16)
    return {"ka": np.ascontiguousarray(ka),
            "vp": np.ascontiguousarray(vp),
            "wp": np.ascontiguousarray(wp) if wp is not None else None}


def _prep_core(qp_half, poly, bf16):
    q2 = (qp_half * qp_half).sum(-1)
    one_q = np.ones(QS, np.float32)
    qa5 = np.stack([2 * qp_half[:, 0], 2 * qp_half[:, 1], 2 * qp_half[:, 2],
                    -one_q, -q2]).astype(np.float32)
    qa_hi, qa_lo = _hilo(qa5, bf16)
    qa = np.concatenate([qa_hi, qa_hi, qa_lo])          # [15, QS]
    ut = None
    if poly:
        U = _build_U(qp_half.astype(np.float64))        # [QS, R]
        ut = np.ascontiguousarray(U.T.astype(np.float32)).astype(bf16)
    return {"qa": np.ascontiguousarray(qa), "ut": ut}


def kernel(query_positions, key_positions, values, masked_elements,
           lengthscales, w_out, _want_trace=False):
    import ml_dtypes
    from concourse.bass_utils import run_bass_kernel_spmd

    bf16 = ml_dtypes.bfloat16
    qp = np.asarray(query_positions, np.float32)
    kp = np.asarray(key_positions, np.float32)
    vals = np.asarray(values, np.float32)
    mask = np.asarray(masked_elements).astype(bool)
    ls = np.asarray(lengthscales, np.float32)
    w = np.asarray(w_out, np.float32)

    cv = (1.0 / (ls.astype(np.float64) ** 2)).astype(np.float32)
    maxcnt = int((~mask).sum(1).max())
    KT2 = max(1, -(-maxcnt // 128))
    nc = _build(tuple(float(x) for x in cv), KT2)
    (poly, exp_heads, chain, stream_h, last, last_j, early_heads,
     early_pairs, row_of) = _plan(cv)

    wt = np.ascontiguousarray(w.T).reshape(4, 128, OUTD) \
        .transpose(1, 0, 2).astype(bf16)
    NEH = len(early_heads)
    sel6 = np.zeros((NEH, len(early_pairs), 128), np.float32)
    for idx, j in enumerate(early_pairs):
        sel6[row_of[2 * j], idx, :64] = 1.0
        sel6[row_of[2 * j + 1], idx, 64:] = 1.0
    sel6 = sel6.astype(bf16)

    bprep = [_prep_batch(kp[b], vals[b], mask[b], KT2, poly, bf16)
             for b in range(B)]
    in_maps = []
    for c in range(NCORES):
        b, hf = c // 2, c % 2
        cprep = _prep_core(qp[b, hf * QS:(hf + 1) * QS], poly, bf16)
        m = {"ka": bprep[b]["ka"], "qa": cprep["qa"], "vp": bprep[b]["vp"],
             "wt": wt, "sel6": sel6}
        if poly:
            m["wp"] = bprep[b]["wp"]
            m["ut"] = cprep["ut"]
        in_maps.append(m)
    res = run_bass_kernel_spmd(nc, in_maps, core_ids=list(range(NCORES)),
                               trace=_want_trace)
    out = np.empty((B, LQ, OUTD), np.float32)
    for c in range(NCORES):
        b, hf = c // 2, c % 2
        out[b, hf * QS:(hf + 1) * QS, :] = res.results[c]["outT"].T
    if _want_trace:
        return out, res
    return out
